# revision 35
# baseline (speedup 1.0000x reference)
"""Trainium2 kernel for nn_KeyedLayer: out = (W_sparse @ x.T).T

W is [16384, 16384] sparse COO (rows sorted, ~128 nnz/row, 2M nnz),
x is [64, 16384] fp32.  Strategy: shard output rows across 8 cores
(2048 rows each; disjoint outputs, no collectives).  Each core computes
out.T[2048, 64] = W_core @ x.T as a dense matmul with W densified on
the host in fp8 e3m4 (1.3e-2 rel err, well within tolerance) as the
stationary operand and x fp16 as the moving operand; K=16384 contracted
in 128 blocks through PSUM.

The kernel is DMA-bound: 32 MiB of fp8 W per core at ~360 B/ns (~93 us)
plus 2 MiB x.  W streams in 2 MiB chunks with deep prefetch; the last
chunks are single k-blocks (256 KiB, still above the ~650 ns dispatch
cost) so the post-last-byte matmul burst is short.
"""

import os
from contextlib import ExitStack

import numpy as np
import ml_dtypes

import concourse.bass as bass
import concourse.tile as tile
from concourse import bacc, mybir
from concourse.bass_utils import run_bass_kernel_spmd

B = 64
IN_DIM = 16384
OUT_DIM = 16384
N_CORES = 8
ROWS_PER_CORE = OUT_DIM // N_CORES  # 2048
KBLK = IN_DIM // 128  # 128 k-blocks of 128
NT = ROWS_PER_CORE // 128  # 16 row-tiles of 128 rows

# W stream: KGRP k-blocks per 2 MiB DMA chunk for the bulk, then NTAIL
# single-k-block (256 KiB) chunks at the end.
KGRP = int(os.environ.get("KERNEL_KGRP", "8"))
NTAIL = int(os.environ.get("KERNEL_NTAIL", "24"))
NBULK = (KBLK - NTAIL) // KGRP
assert NBULK * KGRP + NTAIL == KBLK
WBUFS = int(os.environ.get("KERNEL_WBUFS", "8"))

F16 = mybir.dt.float16
FP8 = mybir.dt.float8e3  # e3m4
FP8_NP = ml_dtypes.float8_e3m4
F32 = mybir.dt.float32

_CACHE = {}

LAST_RESULT = None  # BassKernelResults of the most recent run (for test.py)


def _build_program():
    if "nc" in _CACHE:
        return _CACHE["nc"]
    nc = bacc.Bacc(
        "TRN2", target_bir_lowering=False, debug=False, num_devices=N_CORES
    )
    xT_d = nc.dram_tensor("xT", [128, KBLK, B], F16, kind="ExternalInput")
    # wt[k, p, r] = W[base + r, k*128 + p]
    wt_d = nc.dram_tensor("wt", [KBLK, 128, ROWS_PER_CORE], FP8,
                          kind="ExternalInput")
    # out[p, t, b] = out[b, base + t*128 + p] (host untangles the layout)
    out_d = nc.dram_tensor("out", [128, NT, B], F16, kind="ExternalOutput")

    with tile.TileContext(nc) as tc, ExitStack() as ctx:
        xpool = ctx.enter_context(tc.tile_pool(name="x", bufs=1))
        wpool = ctx.enter_context(tc.tile_pool(name="w", bufs=WBUFS))
        tpool = ctx.enter_context(tc.tile_pool(name="wt", bufs=NTAIL + 1))
        opool = ctx.enter_context(tc.tile_pool(name="o", bufs=1))
        pspool = ctx.enter_context(
            tc.tile_pool(name="ps", bufs=1, space=bass.MemorySpace.PSUM)
        )

        xsb = xpool.tile([128, KBLK, B], F16)  # 2 MiB
        nc.sync.dma_start(xsb[:], xT_d[:])

        # out.T as 16 tiles of [128 rows, 64 batch] fp32 = 4 KiB/partition
        psum = pspool.tile([128, NT, B], F32)

        # PSUM "start" zeroes the whole 2 KiB bank (zero region), so exactly
        # one start/stop per bank: 8 row-tiles of [128, 64] fp32 share a bank.
        TPB = 2048 // (B * 4)  # row-tiles per PSUM bank (8)

        def kblock_matmuls(wap, k):
            # wap: [128 (k-part), NT*128 rows] fp8 for this k-block
            for t in range(NT):
                nc.tensor.matmul(
                    psum[:, t, :],
                    wap[:, t * 128:(t + 1) * 128],  # lhsT [128, 128] fp8
                    xsb[:, k, :],                   # rhs  [128, 64] fp16
                    start=(k == 0 and t % TPB == 0),
                    stop=(k == KBLK - 1 and t % TPB == TPB - 1),
                    skip_group_check=True,
                )

        for g in range(NBULK):
            wsb = wpool.tile([128, KGRP, ROWS_PER_CORE], FP8)  # 2 MiB
            nc.sync.dma_start(
                wsb[:],
                wt_d.ap()[g * KGRP:(g + 1) * KGRP]
                .rearrange("g p r -> p g r"),
            )
            for j in range(KGRP):
                kblock_matmuls(wsb[:, j, :], g * KGRP + j)
        for i in range(NTAIL):
            k = NBULK * KGRP + i
            if i < NTAIL - 1:
                wsb = tpool.tile([128, ROWS_PER_CORE], FP8)  # 256 KiB
                nc.sync.dma_start(wsb[:], wt_d[k])
                kblock_matmuls(wsb[:], k)
            else:
                # split the final k-block at the PSUM-bank boundary so bank
                # 0's last matmuls and copyback overlap bank 1's transfer
                half_r = ROWS_PER_CORE // 2
                for hh in range(2):
                    wsb = tpool.tile([128, half_r], FP8)  # 128 KiB
                    nc.sync.dma_start(
                        wsb[:], wt_d[k][:, hh * half_r:(hh + 1) * half_r])
                    for t in range(hh * TPB, (hh + 1) * TPB):
                        nc.tensor.matmul(
                            psum[:, t, :],
                            wsb[:, (t - hh * TPB) * 128:
                                 (t - hh * TPB + 1) * 128],
                            xsb[:, k, :],
                            start=False,
                            stop=(t % TPB == TPB - 1),
                            skip_group_check=True,
                        )

        # PSUM -> SBUF fp16 copyback per bank on DVE, each half stored via
        # an Act-issued DMA so the first store's DGE overlaps the second
        # copy.
        osb = opool.tile([128, NT, B], F16)
        cut = NT - 6
        nc.vector.tensor_copy(osb[:, :cut, :], psum[:, :cut, :])
        nc.scalar.dma_start(out_d.ap()[:, :cut, :], osb[:, :cut, :])
        nc.vector.tensor_copy(osb[:, cut:, :], psum[:, cut:, :])
        nc.scalar.dma_start(out_d.ap()[:, cut:, :], osb[:, cut:, :])

    nc.compile()
    _CACHE["nc"] = nc
    return nc


def kernel(x_affine: np.ndarray, rows: np.ndarray, cols: np.ndarray,
           vals: np.ndarray) -> np.ndarray:
    global LAST_RESULT
    import scipy.sparse as sp

    x_affine = np.asarray(x_affine, dtype=np.float32)
    rows = np.asarray(rows, dtype=np.int64)
    cols = np.asarray(cols, dtype=np.int64)
    vals = np.asarray(vals, dtype=np.float32)

    # xT host layout [p, k, b]: element = x[b, k*128 + p]
    xT = np.ascontiguousarray(
        x_affine.T.reshape(KBLK, 128, B).transpose(1, 0, 2)
    ).astype(np.float16)

    # rows is sorted; slice each core's nnz range and densify only its
    # [16384, 2048] W.T block (duplicates are summed by scipy).
    in_maps = []
    for c in range(N_CORES):
        base = c * ROWS_PER_CORE
        lo, hi = np.searchsorted(rows, [base, base + ROWS_PER_CORE])
        w_slice = sp.coo_matrix(
            (vals[lo:hi], (cols[lo:hi], rows[lo:hi] - base)),
            shape=(IN_DIM, ROWS_PER_CORE),
        ).toarray()  # [16384, 2048] fp32, w_slice[k, r] = W[base+r, k]
        wt = w_slice.astype(FP8_NP).reshape(KBLK, 128, ROWS_PER_CORE)
        in_maps.append({"xT": xT, "wt": wt})

    nc = _build_program()
    res = run_bass_kernel_spmd(
        nc, in_maps, list(range(N_CORES)),
        trace=bool(int(os.environ.get("KERNEL_TRACE", "0"))),
    )
    LAST_RESULT = res
    # out_d[p, t, b] = out[b, base + t*128 + p]
    out = np.empty((B, OUT_DIM), dtype=np.float32)
    for c in range(N_CORES):
        o = res.results[c]["out"].astype(np.float32)  # [128, NT, B]
        out[:, c * ROWS_PER_CORE:(c + 1) * ROWS_PER_CORE] = (
            o.transpose(1, 0, 2).reshape(ROWS_PER_CORE, B).T
        )
    return out
